# revision 44
# baseline (speedup 1.0000x reference)
"""Trainium2 Bass kernel for the HCFDA dense-CNN module (bf16 v3).

Math used (exact reassociations of the reference):
  1. The 256x256 1x1 DCT conv is only consumed through a channel-mean, so
     temp[b,h,w] = sum_c m[c] * x[b,c,h,w]  with  m = dct_w.mean(axis=0).
  2. Each diffusion step's 3x3 reflect-pad conv collapses (equal symmetric
     kernel rows) to  T' = Ghat @ A + G4hat @ T_mid  with A the left+right
     shift sum, Ghat = alpha*a*S + c1*I, G4hat = 4*alpha*a*S + c24*I: the
     elementwise epilogue is folded into the matmul weights.
  3. SE branch: pooled stats -> two tiny FCs -> sigmoid, per reference.
  out = x * sigmoid(att[c] * sigmoid(T3)[h,w])

Implementation (bf16):
  - x streamed + kept in SBUF as bf16 (halves both DMA directions).
  - temp GEMV transposed: per h-row the x chunk [128c,128w] is stationary,
    m the moving column, so temp^T lands w-major on 128 partitions and the
    PSUM egress is a cheap [128,16] copy. The diffusion is transpose-
    symmetric (symmetric kernel, reflect pad, H == W) so it runs on temp^T
    unchanged; only the final 128x128 heat map is transposed back (PE).
  - channel max: DVE TT-max chains at 2x_1p, with the last tile folded
    separately and merged so the post-stream critical path is short.
  - channel sums split by engine capacity: gpsimd TT-add chains (early
    tiles), ACT accum_out copies, DVE STT+accum_out (late tiles).
  - phase B: PE ones-matmul broadcast of heat rows (one upfront heat-row
    DMA), ACT sigmoid (scale=att) PSUM->SBUF bf16, DVE bf16 mul; output
    DMAs dispatched from the otherwise idle gpsimd sequencer so the sync
    queue never blocks the next chunk's work.

Sharding: pure data parallel, one batch element per NeuronCore (B=8).
"""

import numpy as np
from contextlib import ExitStack

B, C, H, W = 8, 256, 128, 128
HW = H * W           # 16384
NCHUNK = 8           # chunks over HW
CH = HW // NCHUNK    # 2048
N_CORES = 8

# channel-sum engine assignment per (t, j) tile: early chunks on ACT
# (accum_out), late chunks on DVE STT. gpsimd is kept idle during the
# input stream: heavy Pool ops there trip the chip power throttle and
# duty-cycle the DVE to ~25%.
ACT_SUM_TILES = {(0, 0), (1, 0), (0, 1), (1, 1), (0, 2), (1, 2),
                 (0, 3), (1, 3), (0, 4), (1, 4)}
# remaining tiles take the DVE STT path

# phase-B tiles whose sigmoid is evaluated as a quadratic polynomial on the
# DVE (via per-partition coefficients) instead of the ACT table. Disabled:
# STT runs at 1x so a poly tile costs ~5.4us of DVE vs 2.06us of ACT.
POLY_TILES = set()


def _reflect(i, n):
    if i < 0:
        return -i
    if i >= n:
        return 2 * (n - 1) - i
    return i


def _build_program(c1, c24, q0, q1, q2):
    from concourse import bass, mybir, tile
    from concourse import bacc

    f32 = mybir.dt.float32
    bf16 = mybir.dt.bfloat16
    AF = mybir.ActivationFunctionType
    ALU = mybir.AluOpType
    AX = mybir.AxisListType

    nc = bacc.Bacc("TRN2", target_bir_lowering=False, debug=False,
                   num_devices=N_CORES)

    xb = nc.dram_tensor("xb", [C, HW], bf16, kind="ExternalInput").ap()
    mv = nc.dram_tensor("mv", [128, 2], bf16, kind="ExternalInput").ap()
    gm = nc.dram_tensor("gm", [128, 128], bf16, kind="ExternalInput").ap()
    gm4 = nc.dram_tensor("gm4", [128, 128], bf16, kind="ExternalInput").ap()
    w1d = nc.dram_tensor("w1t", [128, 32], f32, kind="ExternalInput").ap()
    w2d = nc.dram_tensor("w2t", [16, 256], f32, kind="ExternalInput").ap()
    b1d = nc.dram_tensor("b1c", [16, 1], f32, kind="ExternalInput").ap()
    b2d = nc.dram_tensor("b2c", [128, 2], f32, kind="ExternalInput").ap()
    ond = nc.dram_tensor("onr", [1, 128], bf16, kind="ExternalInput").ap()
    idd = nc.dram_tensor("idm", [128, 128], bf16, kind="ExternalInput").ap()
    outd = nc.dram_tensor("out", [C, HW], bf16, kind="ExternalOutput").ap()

    with tile.TileContext(nc) as tc, ExitStack() as ctx:
        const = ctx.enter_context(tc.tile_pool(name="const", bufs=1))
        xpool = ctx.enter_context(tc.tile_pool(name="xp", bufs=1))
        work = ctx.enter_context(tc.tile_pool(name="work", bufs=2))
        stat = ctx.enter_context(tc.tile_pool(name="stat", bufs=1))
        actx = ctx.enter_context(ExitStack())
        psA = actx.enter_context(tc.tile_pool(name="psA", bufs=2, space="PSUM"))
        psF = actx.enter_context(tc.tile_pool(name="psF", bufs=2, space="PSUM"))

        # ---------- warm the ACT function tables before any real work ----
        wsrc = const.tile([1, 2], f32, tag="wsrc", name="wsrc")
        nc.vector.memset(wsrc[:], 0.25)
        warm = const.tile([1, 2], f32, tag="warm", name="warm")
        nc.scalar.activation(warm[:], wsrc[:], AF.Sigmoid)
        nc.scalar.activation(warm[:], wsrc[:], AF.Relu)

        # ---------- x streaming first: earliest possible DMA start ----------
        xt = {}
        for j in range(NCHUNK):
            for t in range(2):
                xt[t, j] = xpool.tile([128, CH], bf16, tag=f"x{t}_{j}",
                                      name=f"x{t}_{j}")
                nc.sync.dma_start(
                    out=xt[t, j][:],
                    in_=xb[t * 128:(t + 1) * 128, j * CH:(j + 1) * CH])

        m_sb = const.tile([128, 2], bf16, tag="m", name="m")
        nc.sync.dma_start(out=m_sb[:], in_=mv)
        g_sb = const.tile([128, 128], bf16, tag="g", name="g")
        nc.sync.dma_start(out=g_sb[:], in_=gm)
        g4_sb = const.tile([128, 128], bf16, tag="g4", name="g4")
        nc.sync.dma_start(out=g4_sb[:], in_=gm4)
        w1_sb = const.tile([128, 32], f32, tag="w1", name="w1")
        nc.sync.dma_start(out=w1_sb[:], in_=w1d)
        w2_sb = const.tile([16, 256], f32, tag="w2", name="w2")
        nc.sync.dma_start(out=w2_sb[:], in_=w2d)
        b1_sb = const.tile([16, 1], f32, tag="b1", name="b1")
        nc.sync.dma_start(out=b1_sb[:], in_=b1d)
        b2_sb = const.tile([128, 2], f32, tag="b2", name="b2")
        nc.sync.dma_start(out=b2_sb[:], in_=b2d)
        on_sb = const.tile([1, 128], bf16, tag="onr", name="onr")
        nc.sync.dma_start(out=on_sb[:], in_=ond)
        id_sb = const.tile([128, 128], bf16, tag="idm", name="idm")
        nc.sync.dma_start(out=id_sb[:], in_=idd)

        sums = stat.tile([128, 2, 2 * NCHUNK], f32, tag="sums", name="sums")
        nc.vector.memset(sums[:], 0.0)
        junk = [stat.tile([128, CH], bf16, tag=f"junk{t}",
                          name=f"junk{t}") for t in range(2)]
        mx = [stat.tile([128, CH], bf16, tag=f"mx{t}", name=f"mx{t}")
              for t in range(2)]
        Tp = [stat.tile([128, W + 2], bf16, tag=f"Tp{i}", name=f"Tp{i}")
              for i in range(4)]
        heat = stat.tile([128, W], bf16, tag="heat", name="heat")
        ysb = stat.tile([128, 2, 2], f32, tag="ysb", name="ysb")  # [avg,max]
        pf1 = stat.tile([128, 2, 1024], bf16, tag="pf1", name="pf1")
        pf2 = stat.tile([128, 2, 512], bf16, tag="pf2", name="pf2")
        lf1 = stat.tile([128, 2, 1024], bf16, tag="lf1", name="lf1")
        lf2 = stat.tile([128, 2, 512], bf16, tag="lf2", name="lf2")
        ysum = stat.tile([128, 2], f32, tag="ysum", name="ysum")
        junks = stat.tile([128, 2 * NCHUNK], f32, tag="junks", name="junks")

        # ---------- Phase A ----------
        last = NCHUNK - 1
        for j in range(NCHUNK):
            # temp^T GEMV: psT[:, r] = sum_c m[c] * x[c, 128r:128r+128]
            psT = psA.tile([128, 16], f32, tag="psA", name="psA")
            for t in range(2):
                for r in range(16):
                    nc.tensor.matmul(
                        psT[:, r:r + 1],
                        xt[t, j][:, r * 128:(r + 1) * 128],
                        m_sb[:, t:t + 1],
                        start=(t == 0), stop=(t == 1))
            # late chunks' PSUM egress on DVE: the ACT queue still holds
            # ~2us-per-tile accum sums and would stall the GEMV's psA
            # double-buffer ping-pong
            if j >= 6:
                nc.vector.tensor_copy(Tp[0][:, 1 + 16 * j:1 + 16 * j + 16],
                                      psT[:])
            else:
                nc.scalar.copy(Tp[0][:, 1 + 16 * j:1 + 16 * j + 16], psT[:])

            for t in range(2):
                # channel sums
                if (t, j) in ACT_SUM_TILES:
                    nc.scalar.activation(junk[t][:], xt[t, j][:], AF.Copy,
                                         accum_out=sums[:, t, 2 * j:2 * j + 1])
                else:
                    nc.vector.scalar_tensor_tensor(
                        junk[t][:, 0:CH // 2], xt[t, j][:, 0:CH // 2], 1.0,
                        xt[t, j][:, CH // 2:CH],
                        op0=ALU.mult, op1=ALU.add,
                        accum_out=sums[:, t, 2 * j:2 * j + 1])
                # max chains: pair-init, then links; last tile folds aside
                if j == 1:
                    nc.vector.tensor_tensor(mx[t][:], xt[t, 0][:],
                                            xt[t, 1][:], op=ALU.max)
                elif 2 <= j <= last - 1:
                    nc.vector.tensor_tensor(mx[t][:], mx[t][:], xt[t, j][:],
                                            op=ALU.max)

        # ---------- pooled stats finalize (short tail after last tile) ----
        # ysum/yavg finalize runs on ACT (accumulate trick) to keep the
        # drained-last DVE queue short
        for t in range(2):
            nc.vector.tensor_tensor(pf1[:, t, :], mx[t][:, 0:1024],
                                    mx[t][:, 1024:2048], op=ALU.max)
            nc.vector.tensor_tensor(pf2[:, t, :], pf1[:, t, 0:512],
                                    pf1[:, t, 512:1024], op=ALU.max)
            nc.vector.tensor_tensor(lf1[:, t, :], xt[t, last][:, 0:1024],
                                    xt[t, last][:, 1024:2048], op=ALU.max)
            nc.vector.tensor_tensor(lf2[:, t, :], lf1[:, t, 0:512],
                                    lf1[:, t, 512:1024], op=ALU.max)
            nc.vector.tensor_tensor(pf2[:, t, :], pf2[:, t, :],
                                    lf2[:, t, :], op=ALU.max)
            nc.vector.reduce_max(ysb[:, t, 1:2], pf2[:, t, :], axis=AX.X)
            nc.scalar.activation(junks[:], sums[:, t, :], AF.Copy,
                                 accum_out=ysum[:, t:t + 1])
            nc.scalar.mul(ysb[:, t, 0:1], ysum[:, t:t + 1], 1.0 / HW)

        # ---------- diffusion: 3 steps, epilogue folded into Ghat/G4hat --
        # elementwise ops kept OFF the DVE (its queue drains the pooled-
        # stats backlog): shift-adds + pad copies on gpsimd (post-stream,
        # no throttle risk), PSUM egress on ACT.
        nc.scalar.copy(Tp[0][:, 0:1], Tp[0][:, 2:3])
        nc.scalar.copy(Tp[0][:, W + 1:W + 2], Tp[0][:, W - 1:W])
        psD = actx.enter_context(tc.tile_pool(name="psD", bufs=1,
                                              space="PSUM"))
        for i in range(3):
            cur, nxt = Tp[i], Tp[i + 1]
            A = work.tile([128, W], bf16, tag="dA", name="dA")
            nc.gpsimd.tensor_tensor(A[:], cur[:, 0:W], cur[:, 2:W + 2],
                                    op=ALU.add)
            pd = psD.tile([128, W], f32, tag="psD", name="psD")
            nc.tensor.matmul(pd[:], g_sb[:], A[:], start=True, stop=False)
            nc.tensor.matmul(pd[:], g4_sb[:], cur[:, 1:W + 1],
                             start=False, stop=True)
            nc.scalar.copy(nxt[:, 1:W + 1], pd[:])
            nc.scalar.copy(nxt[:, 0:1], nxt[:, 2:3])
            nc.scalar.copy(nxt[:, W + 1:W + 2], nxt[:, W - 1:W])

        heatT = stat.tile([128, W], bf16, tag="heatT", name="heatT")
        nc.scalar.activation(heatT[:], Tp[3][:, 1:W + 1], AF.Sigmoid)
        ptr = psF.tile([128, 128], bf16, tag="ptr", name="ptr")
        nc.tensor.transpose(ptr[:], heatT[:], id_sb[:])
        nc.scalar.copy(heat[:], ptr[:])
        hrow = stat.tile([1, HW], bf16, tag="hrow", name="hrow")
        nc.sync.dma_start(out=hrow[:], in_=heat[:, :])

        # ---------- SE FC chain (avg+max branches share matmuls, FD=2) ----
        att = stat.tile([128, 2], f32, tag="att", name="att")
        ph = psF.tile([16, 2], f32, tag="psF", name="ph")
        nc.tensor.matmul(ph[:], w1_sb[:, 0:16], ysb[:, 0, :],
                         start=True, stop=False)
        nc.tensor.matmul(ph[:], w1_sb[:, 16:32], ysb[:, 1, :],
                         start=False, stop=True)
        hb = stat.tile([16, 2], f32, tag="hb", name="hb")
        nc.scalar.activation(hb[:], ph[:], AF.Relu, bias=b1_sb[:])
        for t in range(2):
            pa = psF.tile([128, 2], f32, tag="psF", name=f"pa{t}")
            nc.tensor.matmul(pa[:], w2_sb[:, t * 128:(t + 1) * 128],
                             hb[:], start=True, stop=True)
            sg = stat.tile([128, 2], f32, tag=f"sg{t}", name=f"sg{t}")
            nc.scalar.activation(sg[:], pa[:], AF.Sigmoid,
                                 bias=b2_sb[:, t:t + 1])
            nc.vector.tensor_add(att[:, t:t + 1], sg[:, 0:1], sg[:, 1:2])

        # per-partition poly coefficients: sig(att*h) ~ q0 + h*(ca1 + h*ca2)
        # with ca1 = q1*att, ca2 = q2*att^2
        zb = {}
        if POLY_TILES:
            ca1 = stat.tile([128, 2], f32, tag="ca1", name="ca1")
            ca2 = stat.tile([128, 2], f32, tag="ca2", name="ca2")
            nc.vector.tensor_scalar_mul(ca1[:], att[:], float(q1))
            nc.vector.tensor_tensor(ca2[:], att[:], att[:], op=ALU.mult)
            nc.vector.tensor_scalar_mul(ca2[:], ca2[:], float(q2))
            for (t, j) in sorted(POLY_TILES):
                if j not in zb:
                    zb[j] = stat.tile([128, CH], bf16, tag=f"zb{j}",
                                      name=f"zb{j}")
                    nc.gpsimd.partition_broadcast(
                        zb[j][:], hrow[0:1, j * CH:(j + 1) * CH])

        # ---------- Phase B: out = x * sigmoid(att * heat) ----------
        actx.close()  # free phase-A PSUM banks for psB
        with tc.tile_pool(name="psB", bufs=2, space="PSUM") as psB:
            for j in range(NCHUNK):
                if not all((t, j) in POLY_TILES for t in range(2)):
                    pb = psB.tile([128, CH], f32, tag="psB", name="psB")
                    for s in range(4):
                        nc.tensor.matmul(
                            pb[:, s * 512:(s + 1) * 512], on_sb[:],
                            hrow[0:1, j * CH + s * 512:j * CH + (s + 1) * 512],
                            start=True, stop=True)
                nhalf = 2 if j == NCHUNK - 1 else 1
                for t in range(2):
                    o = work.tile([128, CH], bf16, tag="o", name="o",
                                  bufs=3)
                    if (t, j) in POLY_TILES:
                        s1 = work.tile([128, CH], bf16, tag="s1", name="s1",
                                       bufs=2)
                        nc.vector.tensor_scalar(
                            s1[:], zb[j][:], ca2[:, t:t + 1],
                            ca1[:, t:t + 1], op0=ALU.mult, op1=ALU.add)
                        s2 = work.tile([128, CH], bf16, tag="s2", name="s2",
                                       bufs=2)
                        nc.vector.scalar_tensor_tensor(
                            s2[:], s1[:], 1.0, zb[j][:],
                            op0=ALU.mult, op1=ALU.mult)
                        nc.vector.scalar_tensor_tensor(
                            o[:], s2[:], float(q0), xt[t, j][:],
                            op0=ALU.add, op1=ALU.mult)
                        nc.gpsimd.dma_start(
                            out=outd[t * 128:(t + 1) * 128,
                                     j * CH:(j + 1) * CH],
                            in_=o[:])
                    else:
                        sc = work.tile([128, CH], bf16, tag="sc", name="sc",
                                       bufs=3)
                        cw = CH // nhalf
                        for u in range(nhalf):
                            sl = slice(u * cw, (u + 1) * cw)
                            nc.scalar.activation(sc[:, sl], pb[:, sl],
                                                 AF.Sigmoid,
                                                 scale=att[:, t:t + 1])
                            nc.vector.tensor_mul(o[:, sl], xt[t, j][:, sl],
                                                 sc[:, sl])
                            nc.gpsimd.dma_start(
                                out=outd[t * 128:(t + 1) * 128,
                                         j * CH + u * cw:j * CH
                                         + (u + 1) * cw],
                                in_=o[:, sl])

    nc.compile()
    return nc


_prog_cache = {}
_TRACE = False      # test harness sets True to collect an NTFF profile
_last_res = None    # BassKernelResults of the most recent run


def kernel(x, dct_w, w1, b1, w2, b2, alpha, lap):
    import ml_dtypes
    bf = ml_dtypes.bfloat16

    x = np.asarray(x, dtype=np.float32)
    dct_w = np.asarray(dct_w, dtype=np.float32)
    w1 = np.asarray(w1, dtype=np.float32)
    b1 = np.asarray(b1, dtype=np.float32)
    w2 = np.asarray(w2, dtype=np.float32)
    b2 = np.asarray(b2, dtype=np.float32)
    alpha = float(np.asarray(alpha))
    lap = np.asarray(lap, dtype=np.float64)

    # decomposition requires the kernel's row structure (holds for HCFDA's
    # fixed Laplacian); verify.
    assert np.allclose(lap[0], lap[2]) and np.allclose(lap[:, 0], lap[:, 2])
    a, b = float(lap[0, 0]), float(lap[0, 1])
    c1 = alpha * float(lap[1, 0])
    c2 = 1.0 + alpha * (float(lap[1, 1]) - float(lap[1, 0]) * b / a)
    c24 = c2 + 4.0 * c1

    m = dct_w.astype(np.float64).mean(axis=0)           # [C]
    S = np.zeros((H, H), dtype=np.float64)
    for h in range(H):
        S[h, _reflect(h - 1, H)] += 1.0
        S[h, _reflect(h + 1, H)] += 1.0
    G = (alpha * a) * S
    ghat = G + c1 * np.eye(H)         # folds the c1*A term into the matmul
    g4hat = 4.0 * G + c24 * np.eye(H)  # folds c24*T_mid into the matmul

    mv = np.ascontiguousarray(m.reshape(2, 128).T.astype(bf))
    w1t = np.ascontiguousarray(
        w1.T.reshape(2, 128, 16).transpose(1, 0, 2).reshape(128, 32))
    w2t = np.ascontiguousarray(w2.T)                     # [16,256]
    b1c = np.ascontiguousarray(b1.reshape(16, 1))
    b2c = np.ascontiguousarray(b2.reshape(2, 128).T)     # [128,2]

    # near-minimax quadratic fit of sigmoid on z in [0, 2] (Chebyshev)
    zg = np.cos(np.pi * (np.arange(2000) + 0.5) / 2000) + 1.0  # cheb nodes
    cfit = np.polynomial.chebyshev.Chebyshev.fit(
        zg, 1.0 / (1.0 + np.exp(-zg)), deg=2, domain=[0.0, 2.0])
    q0, q1, q2 = (float(c) for c in
                  cfit.convert(kind=np.polynomial.Polynomial).coef)
    pz = np.linspace(0, 2, 4001)
    perr = np.abs(q0 + q1 * pz + q2 * pz * pz - 1.0 / (1.0 + np.exp(-pz)))
    assert perr.max() < 4e-3, perr.max()

    key = (c1, c24, q0, q1, q2)
    if key not in _prog_cache:
        _prog_cache[key] = _build_program(c1, c24, q0, q1, q2)
    nc = _prog_cache[key]

    xbf = x.reshape(B, C, HW).astype(bf)                 # host-side downcast

    consts = {"mv": mv,
              "gm": np.ascontiguousarray(ghat.T.astype(bf)),
              "gm4": np.ascontiguousarray(g4hat.T.astype(bf)),
              "w1t": w1t, "w2t": w2t,
              "b1c": b1c, "b2c": b2c,
              "onr": np.ones((1, 128), dtype=bf),
              "idm": np.eye(128, dtype=bf)}
    in_maps = [{"xb": np.ascontiguousarray(xbf[i]), **consts}
               for i in range(N_CORES)]

    from concourse.bass_utils import run_bass_kernel_spmd
    res = run_bass_kernel_spmd(nc, in_maps, list(range(N_CORES)),
                               trace=_TRACE)
    global _last_res
    _last_res = res
    out = np.stack([res.results[i]["out"].reshape(C, H, W)
                    for i in range(N_CORES)])
    return out.astype(np.float32)


# revision 45
# speedup vs baseline: 1.0271x; 1.0271x over previous
"""Trainium2 Bass kernel for the HCFDA dense-CNN module (bf16 v3).

Math used (exact reassociations of the reference):
  1. The 256x256 1x1 DCT conv is only consumed through a channel-mean, so
     temp[b,h,w] = sum_c m[c] * x[b,c,h,w]  with  m = dct_w.mean(axis=0).
  2. Each diffusion step's 3x3 reflect-pad conv collapses (equal symmetric
     kernel rows) to  T' = Ghat @ A + G4hat @ T_mid  with A the left+right
     shift sum, Ghat = alpha*a*S + c1*I, G4hat = 4*alpha*a*S + c24*I: the
     elementwise epilogue is folded into the matmul weights.
  3. SE branch: pooled stats -> two tiny FCs -> sigmoid, per reference.
  out = x * sigmoid(att[c] * sigmoid(T3)[h,w])

Implementation (bf16):
  - x streamed + kept in SBUF as bf16 (halves both DMA directions).
  - temp GEMV transposed: per h-row the x chunk [128c,128w] is stationary,
    m the moving column, so temp^T lands w-major on 128 partitions and the
    PSUM egress is a cheap [128,16] copy. The diffusion is transpose-
    symmetric (symmetric kernel, reflect pad, H == W) so it runs on temp^T
    unchanged; only the final 128x128 heat map is transposed back (PE).
  - channel max: DVE TT-max chains at 2x_1p, with the last tile folded
    separately and merged so the post-stream critical path is short.
  - channel sums split by engine capacity: gpsimd TT-add chains (early
    tiles), ACT accum_out copies, DVE STT+accum_out (late tiles).
  - phase B: PE ones-matmul broadcast of heat rows (one upfront heat-row
    DMA), ACT sigmoid (scale=att) PSUM->SBUF bf16, DVE bf16 mul; output
    DMAs dispatched from the otherwise idle gpsimd sequencer so the sync
    queue never blocks the next chunk's work.

Sharding: pure data parallel, one batch element per NeuronCore (B=8).
"""

import numpy as np
from contextlib import ExitStack

B, C, H, W = 8, 256, 128, 128
HW = H * W           # 16384
NCHUNK = 8           # chunks over HW
CH = HW // NCHUNK    # 2048
N_CORES = 8

# channel-sum engine assignment per (t, j) tile: early chunks on ACT
# (accum_out), late chunks on DVE STT. gpsimd is kept idle during the
# input stream: heavy Pool ops there trip the chip power throttle and
# duty-cycle the DVE to ~25%.
ACT_SUM_TILES = {(0, 0), (1, 0), (0, 1), (1, 1), (0, 2), (1, 2),
                 (0, 3), (1, 3), (0, 4), (1, 4)}
# remaining tiles take the DVE STT path

# phase-B tiles whose sigmoid is evaluated as a quadratic polynomial on the
# DVE (via per-partition coefficients) instead of the ACT table. Disabled:
# STT runs at 1x so a poly tile costs ~5.4us of DVE vs 2.06us of ACT.
POLY_TILES = set()


def _reflect(i, n):
    if i < 0:
        return -i
    if i >= n:
        return 2 * (n - 1) - i
    return i


def _build_program(c1, c24, q0, q1, q2):
    from concourse import bass, mybir, tile
    from concourse import bacc

    f32 = mybir.dt.float32
    bf16 = mybir.dt.bfloat16
    AF = mybir.ActivationFunctionType
    ALU = mybir.AluOpType
    AX = mybir.AxisListType

    nc = bacc.Bacc("TRN2", target_bir_lowering=False, debug=False,
                   num_devices=N_CORES)

    xb = nc.dram_tensor("xb", [C, HW], bf16, kind="ExternalInput").ap()
    mv = nc.dram_tensor("mv", [128, 2], bf16, kind="ExternalInput").ap()
    gm = nc.dram_tensor("gm", [128, 128], bf16, kind="ExternalInput").ap()
    gm4 = nc.dram_tensor("gm4", [128, 128], bf16, kind="ExternalInput").ap()
    w1d = nc.dram_tensor("w1t", [128, 32], f32, kind="ExternalInput").ap()
    w2d = nc.dram_tensor("w2t", [16, 256], f32, kind="ExternalInput").ap()
    b1d = nc.dram_tensor("b1c", [16, 1], f32, kind="ExternalInput").ap()
    b2d = nc.dram_tensor("b2c", [128, 2], f32, kind="ExternalInput").ap()
    ond = nc.dram_tensor("onr", [1, 128], bf16, kind="ExternalInput").ap()
    idd = nc.dram_tensor("idm", [128, 128], bf16, kind="ExternalInput").ap()
    outd = nc.dram_tensor("out", [C, HW], bf16, kind="ExternalOutput").ap()

    with tile.TileContext(nc) as tc, ExitStack() as ctx:
        const = ctx.enter_context(tc.tile_pool(name="const", bufs=1))
        xpool = ctx.enter_context(tc.tile_pool(name="xp", bufs=1))
        work = ctx.enter_context(tc.tile_pool(name="work", bufs=2))
        stat = ctx.enter_context(tc.tile_pool(name="stat", bufs=1))
        actx = ctx.enter_context(ExitStack())
        psA = actx.enter_context(tc.tile_pool(name="psA", bufs=2, space="PSUM"))
        psF = actx.enter_context(tc.tile_pool(name="psF", bufs=2, space="PSUM"))

        # ---------- warm the ACT function tables before any real work ----
        wsrc = const.tile([1, 2], f32, tag="wsrc", name="wsrc")
        nc.vector.memset(wsrc[:], 0.25)
        warm = const.tile([1, 2], f32, tag="warm", name="warm")
        nc.scalar.activation(warm[:], wsrc[:], AF.Sigmoid)
        nc.scalar.activation(warm[:], wsrc[:], AF.Relu)

        # ---------- x streaming first: earliest possible DMA start ----------
        xt = {}
        for j in range(NCHUNK):
            for t in range(2):
                xt[t, j] = xpool.tile([128, CH], bf16, tag=f"x{t}_{j}",
                                      name=f"x{t}_{j}")
                nc.sync.dma_start(
                    out=xt[t, j][:],
                    in_=xb[t * 128:(t + 1) * 128, j * CH:(j + 1) * CH])

        m_sb = const.tile([128, 2], bf16, tag="m", name="m")
        nc.sync.dma_start(out=m_sb[:], in_=mv)
        g_sb = const.tile([128, 128], bf16, tag="g", name="g")
        nc.sync.dma_start(out=g_sb[:], in_=gm)
        g4_sb = const.tile([128, 128], bf16, tag="g4", name="g4")
        nc.sync.dma_start(out=g4_sb[:], in_=gm4)
        w1_sb = const.tile([128, 32], f32, tag="w1", name="w1")
        nc.sync.dma_start(out=w1_sb[:], in_=w1d)
        w2_sb = const.tile([16, 256], f32, tag="w2", name="w2")
        nc.sync.dma_start(out=w2_sb[:], in_=w2d)
        b1_sb = const.tile([16, 1], f32, tag="b1", name="b1")
        nc.sync.dma_start(out=b1_sb[:], in_=b1d)
        b2_sb = const.tile([128, 2], f32, tag="b2", name="b2")
        nc.sync.dma_start(out=b2_sb[:], in_=b2d)
        on_sb = const.tile([1, 128], bf16, tag="onr", name="onr")
        nc.sync.dma_start(out=on_sb[:], in_=ond)
        id_sb = const.tile([128, 128], bf16, tag="idm", name="idm")
        nc.sync.dma_start(out=id_sb[:], in_=idd)

        sums = stat.tile([128, 2, 2 * NCHUNK], f32, tag="sums", name="sums")
        nc.vector.memset(sums[:], 0.0)
        junk = [stat.tile([128, CH], bf16, tag=f"junk{t}",
                          name=f"junk{t}") for t in range(2)]
        mx = [stat.tile([128, CH], bf16, tag=f"mx{t}", name=f"mx{t}")
              for t in range(2)]
        Tp = [stat.tile([128, W + 2], bf16, tag=f"Tp{i}", name=f"Tp{i}")
              for i in range(4)]
        heat = stat.tile([128, W], bf16, tag="heat", name="heat")
        ysb = stat.tile([128, 2, 2], f32, tag="ysb", name="ysb")  # [avg,max]
        pf1 = stat.tile([128, 2, 1024], bf16, tag="pf1", name="pf1")
        pf2 = stat.tile([128, 2, 512], bf16, tag="pf2", name="pf2")
        lf1 = stat.tile([128, 2, 1024], bf16, tag="lf1", name="lf1")
        lf2 = stat.tile([128, 2, 512], bf16, tag="lf2", name="lf2")
        ysum = stat.tile([128, 2], f32, tag="ysum", name="ysum")
        junks = stat.tile([128, 2 * NCHUNK], f32, tag="junks", name="junks")

        # ---------- Phase A ----------
        last = NCHUNK - 1
        for j in range(NCHUNK):
            # temp^T GEMV: psT[:, r] = sum_c m[c] * x[c, 128r:128r+128]
            psT = psA.tile([128, 16], f32, tag="psA", name="psA")
            for t in range(2):
                for r in range(16):
                    nc.tensor.matmul(
                        psT[:, r:r + 1],
                        xt[t, j][:, r * 128:(r + 1) * 128],
                        m_sb[:, t:t + 1],
                        start=(t == 0), stop=(t == 1))
            nc.scalar.copy(Tp[0][:, 1 + 16 * j:1 + 16 * j + 16], psT[:])

            for t in range(2):
                # channel sums
                if (t, j) in ACT_SUM_TILES:
                    nc.scalar.activation(junk[t][:], xt[t, j][:], AF.Copy,
                                         accum_out=sums[:, t, 2 * j:2 * j + 1])
                else:
                    nc.vector.scalar_tensor_tensor(
                        junk[t][:, 0:CH // 2], xt[t, j][:, 0:CH // 2], 1.0,
                        xt[t, j][:, CH // 2:CH],
                        op0=ALU.mult, op1=ALU.add,
                        accum_out=sums[:, t, 2 * j:2 * j + 1])
                # max chains: pair-init, then links; last tile folds aside
                if j == 1:
                    nc.vector.tensor_tensor(mx[t][:], xt[t, 0][:],
                                            xt[t, 1][:], op=ALU.max)
                elif 2 <= j <= last - 1:
                    nc.vector.tensor_tensor(mx[t][:], mx[t][:], xt[t, j][:],
                                            op=ALU.max)

        # ---------- pooled stats finalize (short tail after last tile) ----
        # ysum/yavg finalize runs on ACT (accumulate trick) to keep the
        # drained-last DVE queue short
        for t in range(2):
            nc.vector.tensor_tensor(pf1[:, t, :], mx[t][:, 0:1024],
                                    mx[t][:, 1024:2048], op=ALU.max)
            nc.vector.tensor_tensor(pf2[:, t, :], pf1[:, t, 0:512],
                                    pf1[:, t, 512:1024], op=ALU.max)
            nc.vector.tensor_tensor(lf1[:, t, :], xt[t, last][:, 0:1024],
                                    xt[t, last][:, 1024:2048], op=ALU.max)
            nc.vector.tensor_tensor(lf2[:, t, :], lf1[:, t, 0:512],
                                    lf1[:, t, 512:1024], op=ALU.max)
            nc.vector.tensor_tensor(pf2[:, t, :], pf2[:, t, :],
                                    lf2[:, t, :], op=ALU.max)
            nc.vector.reduce_max(ysb[:, t, 1:2], pf2[:, t, :], axis=AX.X)
            nc.scalar.activation(junks[:], sums[:, t, :], AF.Copy,
                                 accum_out=ysum[:, t:t + 1])
            nc.scalar.mul(ysb[:, t, 0:1], ysum[:, t:t + 1], 1.0 / HW)

        # ---------- diffusion: 3 steps, epilogue folded into Ghat/G4hat --
        # elementwise ops kept OFF the DVE (its queue drains the pooled-
        # stats backlog): shift-adds + pad copies on gpsimd (post-stream,
        # no throttle risk), PSUM egress on ACT.
        nc.scalar.copy(Tp[0][:, 0:1], Tp[0][:, 2:3])
        nc.scalar.copy(Tp[0][:, W + 1:W + 2], Tp[0][:, W - 1:W])
        psD = actx.enter_context(tc.tile_pool(name="psD", bufs=1,
                                              space="PSUM"))
        for i in range(3):
            cur, nxt = Tp[i], Tp[i + 1]
            A = work.tile([128, W], bf16, tag="dA", name="dA")
            nc.gpsimd.tensor_tensor(A[:], cur[:, 0:W], cur[:, 2:W + 2],
                                    op=ALU.add)
            pd = psD.tile([128, W], f32, tag="psD", name="psD")
            nc.tensor.matmul(pd[:], g_sb[:], A[:], start=True, stop=False)
            nc.tensor.matmul(pd[:], g4_sb[:], cur[:, 1:W + 1],
                             start=False, stop=True)
            nc.scalar.copy(nxt[:, 1:W + 1], pd[:])
            nc.scalar.copy(nxt[:, 0:1], nxt[:, 2:3])
            nc.scalar.copy(nxt[:, W + 1:W + 2], nxt[:, W - 1:W])

        heatT = stat.tile([128, W], bf16, tag="heatT", name="heatT")
        nc.scalar.activation(heatT[:], Tp[3][:, 1:W + 1], AF.Sigmoid)
        ptr = psF.tile([128, 128], bf16, tag="ptr", name="ptr")
        nc.tensor.transpose(ptr[:], heatT[:], id_sb[:])
        nc.scalar.copy(heat[:], ptr[:])
        hrow = stat.tile([1, HW], bf16, tag="hrow", name="hrow")
        nc.sync.dma_start(out=hrow[:], in_=heat[:, :])

        # ---------- SE FC chain (avg+max branches share matmuls, FD=2) ----
        att = stat.tile([128, 2], f32, tag="att", name="att")
        ph = psF.tile([16, 2], f32, tag="psF", name="ph")
        nc.tensor.matmul(ph[:], w1_sb[:, 0:16], ysb[:, 0, :],
                         start=True, stop=False)
        nc.tensor.matmul(ph[:], w1_sb[:, 16:32], ysb[:, 1, :],
                         start=False, stop=True)
        hb = stat.tile([16, 2], f32, tag="hb", name="hb")
        nc.scalar.activation(hb[:], ph[:], AF.Relu, bias=b1_sb[:])
        for t in range(2):
            pa = psF.tile([128, 2], f32, tag="psF", name=f"pa{t}")
            nc.tensor.matmul(pa[:], w2_sb[:, t * 128:(t + 1) * 128],
                             hb[:], start=True, stop=True)
            sg = stat.tile([128, 2], f32, tag=f"sg{t}", name=f"sg{t}")
            nc.scalar.activation(sg[:], pa[:], AF.Sigmoid,
                                 bias=b2_sb[:, t:t + 1])
            nc.vector.tensor_add(att[:, t:t + 1], sg[:, 0:1], sg[:, 1:2])

        # per-partition poly coefficients: sig(att*h) ~ q0 + h*(ca1 + h*ca2)
        # with ca1 = q1*att, ca2 = q2*att^2
        zb = {}
        if POLY_TILES:
            ca1 = stat.tile([128, 2], f32, tag="ca1", name="ca1")
            ca2 = stat.tile([128, 2], f32, tag="ca2", name="ca2")
            nc.vector.tensor_scalar_mul(ca1[:], att[:], float(q1))
            nc.vector.tensor_tensor(ca2[:], att[:], att[:], op=ALU.mult)
            nc.vector.tensor_scalar_mul(ca2[:], ca2[:], float(q2))
            for (t, j) in sorted(POLY_TILES):
                if j not in zb:
                    zb[j] = stat.tile([128, CH], bf16, tag=f"zb{j}",
                                      name=f"zb{j}")
                    nc.gpsimd.partition_broadcast(
                        zb[j][:], hrow[0:1, j * CH:(j + 1) * CH])

        # ---------- Phase B: out = x * sigmoid(att * heat) ----------
        actx.close()  # free phase-A PSUM banks for psB
        with tc.tile_pool(name="psB", bufs=2, space="PSUM") as psB:
            for j in range(NCHUNK):
                if not all((t, j) in POLY_TILES for t in range(2)):
                    pb = psB.tile([128, CH], f32, tag="psB", name="psB")
                    for s in range(4):
                        nc.tensor.matmul(
                            pb[:, s * 512:(s + 1) * 512], on_sb[:],
                            hrow[0:1, j * CH + s * 512:j * CH + (s + 1) * 512],
                            start=True, stop=True)
                nhalf = 2 if j == NCHUNK - 1 else 1
                for t in range(2):
                    o = work.tile([128, CH], bf16, tag="o", name="o",
                                  bufs=3)
                    if (t, j) in POLY_TILES:
                        s1 = work.tile([128, CH], bf16, tag="s1", name="s1",
                                       bufs=2)
                        nc.vector.tensor_scalar(
                            s1[:], zb[j][:], ca2[:, t:t + 1],
                            ca1[:, t:t + 1], op0=ALU.mult, op1=ALU.add)
                        s2 = work.tile([128, CH], bf16, tag="s2", name="s2",
                                       bufs=2)
                        nc.vector.scalar_tensor_tensor(
                            s2[:], s1[:], 1.0, zb[j][:],
                            op0=ALU.mult, op1=ALU.mult)
                        nc.vector.scalar_tensor_tensor(
                            o[:], s2[:], float(q0), xt[t, j][:],
                            op0=ALU.add, op1=ALU.mult)
                        nc.gpsimd.dma_start(
                            out=outd[t * 128:(t + 1) * 128,
                                     j * CH:(j + 1) * CH],
                            in_=o[:])
                    else:
                        sc = work.tile([128, CH], bf16, tag="sc", name="sc",
                                       bufs=3)
                        cw = CH // nhalf
                        for u in range(nhalf):
                            sl = slice(u * cw, (u + 1) * cw)
                            nc.scalar.activation(sc[:, sl], pb[:, sl],
                                                 AF.Sigmoid,
                                                 scale=att[:, t:t + 1])
                            nc.vector.tensor_mul(o[:, sl], xt[t, j][:, sl],
                                                 sc[:, sl])
                            nc.gpsimd.dma_start(
                                out=outd[t * 128:(t + 1) * 128,
                                         j * CH + u * cw:j * CH
                                         + (u + 1) * cw],
                                in_=o[:, sl])

    nc.compile()
    return nc


_prog_cache = {}
_TRACE = False      # test harness sets True to collect an NTFF profile
_last_res = None    # BassKernelResults of the most recent run


def kernel(x, dct_w, w1, b1, w2, b2, alpha, lap):
    import ml_dtypes
    bf = ml_dtypes.bfloat16

    x = np.asarray(x, dtype=np.float32)
    dct_w = np.asarray(dct_w, dtype=np.float32)
    w1 = np.asarray(w1, dtype=np.float32)
    b1 = np.asarray(b1, dtype=np.float32)
    w2 = np.asarray(w2, dtype=np.float32)
    b2 = np.asarray(b2, dtype=np.float32)
    alpha = float(np.asarray(alpha))
    lap = np.asarray(lap, dtype=np.float64)

    # decomposition requires the kernel's row structure (holds for HCFDA's
    # fixed Laplacian); verify.
    assert np.allclose(lap[0], lap[2]) and np.allclose(lap[:, 0], lap[:, 2])
    a, b = float(lap[0, 0]), float(lap[0, 1])
    c1 = alpha * float(lap[1, 0])
    c2 = 1.0 + alpha * (float(lap[1, 1]) - float(lap[1, 0]) * b / a)
    c24 = c2 + 4.0 * c1

    m = dct_w.astype(np.float64).mean(axis=0)           # [C]
    S = np.zeros((H, H), dtype=np.float64)
    for h in range(H):
        S[h, _reflect(h - 1, H)] += 1.0
        S[h, _reflect(h + 1, H)] += 1.0
    G = (alpha * a) * S
    ghat = G + c1 * np.eye(H)         # folds the c1*A term into the matmul
    g4hat = 4.0 * G + c24 * np.eye(H)  # folds c24*T_mid into the matmul

    mv = np.ascontiguousarray(m.reshape(2, 128).T.astype(bf))
    w1t = np.ascontiguousarray(
        w1.T.reshape(2, 128, 16).transpose(1, 0, 2).reshape(128, 32))
    w2t = np.ascontiguousarray(w2.T)                     # [16,256]
    b1c = np.ascontiguousarray(b1.reshape(16, 1))
    b2c = np.ascontiguousarray(b2.reshape(2, 128).T)     # [128,2]

    # near-minimax quadratic fit of sigmoid on z in [0, 2] (Chebyshev)
    zg = np.cos(np.pi * (np.arange(2000) + 0.5) / 2000) + 1.0  # cheb nodes
    cfit = np.polynomial.chebyshev.Chebyshev.fit(
        zg, 1.0 / (1.0 + np.exp(-zg)), deg=2, domain=[0.0, 2.0])
    q0, q1, q2 = (float(c) for c in
                  cfit.convert(kind=np.polynomial.Polynomial).coef)
    pz = np.linspace(0, 2, 4001)
    perr = np.abs(q0 + q1 * pz + q2 * pz * pz - 1.0 / (1.0 + np.exp(-pz)))
    assert perr.max() < 4e-3, perr.max()

    key = (c1, c24, q0, q1, q2)
    if key not in _prog_cache:
        _prog_cache[key] = _build_program(c1, c24, q0, q1, q2)
    nc = _prog_cache[key]

    xbf = x.reshape(B, C, HW).astype(bf)                 # host-side downcast

    consts = {"mv": mv,
              "gm": np.ascontiguousarray(ghat.T.astype(bf)),
              "gm4": np.ascontiguousarray(g4hat.T.astype(bf)),
              "w1t": w1t, "w2t": w2t,
              "b1c": b1c, "b2c": b2c,
              "onr": np.ones((1, 128), dtype=bf),
              "idm": np.eye(128, dtype=bf)}
    in_maps = [{"xb": np.ascontiguousarray(xbf[i]), **consts}
               for i in range(N_CORES)]

    from concourse.bass_utils import run_bass_kernel_spmd
    res = run_bass_kernel_spmd(nc, in_maps, list(range(N_CORES)),
                               trace=_TRACE)
    global _last_res
    _last_res = res
    out = np.stack([res.results[i]["out"].reshape(C, H, W)
                    for i in range(N_CORES)])
    return out.astype(np.float32)
